# revision 58
# baseline (speedup 1.0000x reference)
"""MoE actor kernel for 8 TRN2 NeuronCores (expert-parallel, host routing).

Problem: B=65536 tokens, obs dim D=376, each routed by `o` to one of E=8
experts; per-expert MLP 376 -> 256 -> 256 -> {mean[17], log_std[17]} with
relu/relu/(identity|tanh-affine) heads.

Strategy: routing/gather happens on the host (numpy) — core e receives
exactly the tokens assigned to expert e (padded to 512-token tiles plus one
short remainder tile) and only that expert's weights. Every core runs the
same dense 3-layer MLP graph with features on the partition axis:

    h1T[H, n] = relu(W1.T @ xT)          K=384(pad of 376+ones row) -> M=256
    h2T[H, n] = relu(W2.T @ h1T + b2)    K=256 -> M=256
    zT[.., n] = Wc.T @ h2T               K=256 -> M=128 (mean @0:17, z @32:49)
    rows 32:49 -> tanh(z + bs) on ScalarE

b1 rides inside the L1 matmul as an extra K row (x carries a ones row), so
the h1 ReLU is a single bias-free DVE op over a two-bank PSUM tile.
Matmuls run in bf16 (full PE rate, fp32 PSUM accumulate; ~4e-3 rel err).
The loop is software-pipelined three deep — the PE stream is
[L1(t), L2(t-1), L3(t-2)] so activations of one tile always overlap ~2.6us
of independent matmul work and the PE never stalls on ReLU latency.
PSUM: p1 double-buffered (2x2 banks) + p2 (2) + p3 (2) = 8 banks.

DMA lessons baked in: x loads ride sync's hardware DGE ring, batched
XGRP tiles per buffer with multi-KB contiguous rows (~355GB/s vs ~130 for
single-tile rows; two half-group transfers per buffer for finer completion
granularity), and group buffers rotate (bufs=3) so transfers don't compete
for ring bandwidth. Stores only sustain ~25-40GB/s on ANY ring, so outputs
are bf16 and split across rings: mean chunks on gpsimd's software ring,
z chunks on sync (program order defers them behind the whole x stream).
Three f32 warm-up matmuls before the first x tile lands keep the HAM
activity monitor fed so the full 2.4GHz clock grant arrives early; with
fewer, the PE can settle at ~2.0GHz for the entire run.

The host scatters per-core outputs back to original token order, adding
the mean bias and the log-std affine (3.5*t - 1.5) during the scatter.
"""

import numpy as np

B, D, H, A, E = 65536, 376, 256, 17, 8
DPAD = 384          # D padded to 3 partition tiles of 128
TOK = 512           # token tile (matmul free dim; one PSUM bank)
AOUT = 2 * A        # 34: mean ++ log_std
CHUNK = 5           # out-DMA batching (tiles per writeback)
XGRP = 8            # x-load batching (tiles per input DMA)
XSOLO = 3           # leading tiles loaded individually (fast start)

# test.py hooks: set TRACE=True before calling kernel() to profile; the
# BassKernelResults of the last run lands in LAST_RESULT.
TRACE = False
TRACE_CORES = None
LAST_RESULT = None

_cache = {}


def _sizes(n_full, rem):
    s = [TOK] * n_full + ([rem] if rem else [])
    offs = np.concatenate([[0], np.cumsum(s)[:-1]]).tolist()
    return s, offs


def _install_axon_ntff_hook():
    """antenv.axon_hooks is absent in this image; recreate it so
    run_bass_kernel_spmd(trace=True) can capture NTFF profiles."""
    import sys, types
    if 'antenv.axon_hooks' in sys.modules:
        return
    try:
        from trn_agent_boot.trn_boot import _ntff_profile_via_ctypes
        hook = _ntff_profile_via_ctypes('/opt/axon/libaxon_pjrt.so')
    except Exception:
        hook = None
    m = types.ModuleType('antenv.axon_hooks')
    m.get_axon_ntff_profile_hook = lambda: hook
    m.set_axon_ntff_profile_hook = lambda h: None
    sys.modules['antenv.axon_hooks'] = m


def _build(n_full, rem):
    import concourse.bass as bass
    import concourse.tile as tile
    from concourse import bacc, mybir

    f32 = mybir.dt.float32
    bf16 = mybir.dt.bfloat16
    AF = mybir.ActivationFunctionType
    ds = bass.ds
    sizes, offs = _sizes(n_full, rem)
    T = len(sizes)
    npad = n_full * TOK + rem

    nc = bacc.Bacc("TRN2", target_bir_lowering=False, debug=False, num_devices=E)
    x_ext = nc.dram_tensor("x", [128, 3 * npad], bf16, kind="ExternalInput")
    w1_ext = nc.dram_tensor("w1", [128, 3 * H], bf16, kind="ExternalInput")
    rest_ext = nc.dram_tensor("rest", [128, 2 * H + 256], bf16,
                              kind="ExternalInput")
    bias_ext = nc.dram_tensor("bias", [128, 3], f32, kind="ExternalInput")
    out_ext = nc.dram_tensor("out", [AOUT, npad], bf16, kind="ExternalOutput")

    with tile.TileContext(nc) as tc:
        with tc.tile_pool(name="wp", bufs=1) as wp, \
             tc.tile_pool(name="xp", bufs=4) as xp, \
             tc.tile_pool(name="xgp", bufs=3) as xgp, \
             tc.tile_pool(name="hp", bufs=4) as hp, \
             tc.tile_pool(name="op", bufs=1) as op, \
             tc.tile_pool(name="ps1", bufs=2, space="PSUM") as ps1, \
             tc.tile_pool(name="ps2", bufs=1, space="PSUM") as ps2, \
             tc.tile_pool(name="ps3", bufs=2, space="PSUM") as ps3:
            # Small warm tiles; memset is quick so the PE's warm-up matmuls
            # (which open the HAM full-clock activity window) start as soon
            # as the prologue ends, bridging until the first x tile lands.
            warm_w = wp.tile([128, 128], f32, name="warm_w")
            nc.gpsimd.memset(warm_w[:], 0.0)
            warm_x = wp.tile([128, TOK], f32, name="warm_x")
            nc.vector.memset(warm_x[:], 0.0)

            # Weight/bias tiles; DMAs split across engine queues so the
            # first L1 matmul only waits for w1 chunk 0 + x piece 0.
            w1k = [wp.tile([128, H], bf16, name=f"w1k{k}") for k in range(3)]
            rest = wp.tile([128, 2 * H + 256], bf16)
            bias = wp.tile([128, 3], f32)
            w2 = rest[:, ds(0, 2 * H)]
            wc = rest[:, ds(2 * H, 256)]
            b2 = bias[:, ds(0, 2)]
            bc = bias[:, ds(2, 1)]

            nc.scalar.dma_start(w1k[0][:], w1_ext.ap()[:, 0:H])
            nc.gpsimd.dma_start(w1k[2][:], w1_ext.ap()[:, 2 * H:3 * H])
            nc.scalar.dma_start(rest[:], rest_ext.ap()[:])
            nc.gpsimd.dma_start(bias[:], bias_ext.ap()[:])

            # PE pre-warm at pre-boost clock while DMAs stream in: keeps the
            # HAM activity window open so the real matmuls run at full clock.
            for _ in range(3):
                pw = ps3.tile([128, TOK], f32, tag="p3", name="pwarm")
                nc.tensor.matmul(pw[:], warm_w[:], warm_x[:],
                                 start=True, stop=True)

            ocs = {}

            def do_l2(t, h1):
                n = sizes[t]
                p2 = [ps2.tile([128, TOK], f32, tag=f"p2_{m}", name=f"p2_{m}")
                      for m in range(2)]
                for k in range(2):
                    for m in range(2):
                        nc.tensor.matmul(
                            p2[m][:, 0:n], w2[:, ds(k * H + m * 128, 128)],
                            h1[:, ds(k * TOK, n)], start=(k == 0),
                            stop=(k == 1))
                h2 = []
                for m in range(2):
                    h = hp.tile([128, TOK], bf16, tag=f"h2_{m}")
                    nc.scalar.activation(h[:, 0:n], p2[m][:, 0:n], AF.Relu,
                                         bias=b2[:, ds(m, 1)])
                    h2.append(h)
                return h2

            def do_ep(t, h2):
                # L3 + epilogue for tile t. Mean rows land in psum 0:17 and
                # leave raw (host adds bm); z rows land in psum 17:34 and get
                # tanh(z + bs) here, the affine on the host. Results gather
                # in a per-chunk SBUF buffer, written back every CHUNK tiles.
                n = sizes[t]
                ci = t // CHUNK
                base = offs[ci * CHUNK]
                loc = offs[t] - base
                if ci not in ocs:
                    # Engine APs must start at partition 0/32/64/96, so z
                    # lives at rows 32:49; the writeback DMAs (partition
                    # start unrestricted) compact to out rows 17:34.
                    # bf16 halves the store bytes (stores only sustain
                    # ~25-40GB/s on any ring).
                    ocs[ci] = op.tile([49, CHUNK * TOK], bf16,
                                      tag=f"oc{ci}", name=f"oc{ci}")
                oc = ocs[ci]
                p3 = ps3.tile([128, TOK], f32, tag="p3")
                for k in range(2):
                    nc.tensor.matmul(
                        p3[:, 0:n], wc[:, ds(k * 128, 128)], h2[k][:, 0:n],
                        start=(k == 0), stop=(k == 1))
                nc.vector.tensor_copy(oc[0:A, loc:loc + n], p3[0:A, 0:n])
                nc.scalar.activation(oc[32:32 + A, loc:loc + n],
                                     p3[32:32 + A, 0:n], AF.Tanh,
                                     bias=bc[32:32 + A, :])
                if t == T - 1 or (t + 1) % CHUNK == 0:
                    # Stores split across both rings: mean on gpsimd's
                    # software ring (flows from early on), z on sync's
                    # hardware ring (program order defers it past the x
                    # stream so they never compete).
                    width = offs[t] + n - base
                    nc.gpsimd.dma_start(out_ext.ap()[0:A, base:base + width],
                                        oc[0:A, 0:width])
                    nc.sync.dma_start(
                        out_ext.ap()[A:AOUT, base:base + width],
                        oc[32:32 + A, 0:width])

            # All x loads are emitted upfront in consumption order; the
            # tile-pool WAR events pace the actual issues, and anything
            # emitted later on sync (the z writebacks) naturally queues
            # behind the whole x stream.
            xk_of = {}
            for t, n in enumerate(sizes):
                xoff = 3 * offs[t]
                if t < XSOLO or n != TOK:
                    xsb = xp.tile([128, 3 * TOK], bf16, tag="x", name="xsb")
                    if t == 0:
                        # Split the first transfer so matmul k only waits
                        # on piece k; w1 chunk 1 rides between the pieces.
                        for k in range(3):
                            nc.sync.dma_start(
                                xsb[:, ds(k * n, n)],
                                x_ext.ap()[:, xoff + k * n:xoff + (k + 1) * n])
                            if k == 0:
                                nc.sync.dma_start(w1k[1][:],
                                                  w1_ext.ap()[:, H:2 * H])
                    else:
                        nc.sync.dma_start(xsb[:, 0:3 * n],
                                          x_ext.ap()[:, xoff:xoff + 3 * n])
                    xk_of[t] = [xsb[:, ds(k * n, n)] for k in range(3)]
                elif t not in xk_of:
                    # Full tiles load in groups: multi-KB contiguous rows
                    # per transfer run the DGE ring at ~355GB/s vs ~130 for
                    # single-tile rows. Two half-group transfers per buffer
                    # give finer completion granularity (less boundary
                    # stall) at the same row width.
                    g = min(XGRP, n_full - t)
                    xgt = xgp.tile([128, 3 * TOK * XGRP], bf16,
                                   tag="xg", name="xg")
                    for h in range(0, g, 2):
                        sub = min(2, g - h)
                        nc.sync.dma_start(
                            xgt[:, ds(3 * TOK * h, 3 * TOK * sub)],
                            x_ext.ap()[:, xoff + 3 * TOK * h:
                                       xoff + 3 * TOK * (h + sub)])
                    for j in range(g):
                        xk_of[t + j] = [
                            xgt[:, ds((3 * j + k) * TOK, TOK)]
                            for k in range(3)]

            stage2 = []   # (t, h1) awaiting L2
            stage3 = []   # (t, h2) awaiting L3/epilogue
            for t, n in enumerate(sizes):
                xk = xk_of[t]

                # p1 spans two PSUM banks so one DVE op can ReLU both
                # halves; the b1 bias rides in the matmul via the x
                # ones-row, so the ReLU needs no per-half bias AP.
                p1 = ps1.tile([128, 2 * TOK], f32, tag="p1", name="p1")
                if t == 0:
                    km_order = [(k, m) for k in range(3) for m in range(2)]
                else:
                    km_order = [(k, m) for m in range(2) for k in range(3)]
                for k, m in km_order:
                    nc.tensor.matmul(
                        p1[:, ds(m * TOK, n)], w1k[k][:, ds(m * 128, 128)],
                        xk[k], start=(k == 0), stop=(k == 2))

                # L2 of the previous tile is emitted (and its ReLUs queued)
                # before this tile's ReLUs so the engine queues drain in
                # dependency-arrival order.
                if stage2:
                    l2_args = stage2.pop(0)
                    stage3.append((l2_args[0], do_l2(*l2_args)))

                h1 = hp.tile([128, 2 * TOK], bf16, tag="h1", name="h1")
                if n == TOK:
                    nc.vector.tensor_scalar(
                        out=h1[:], in0=p1[:], scalar1=0.0, scalar2=None,
                        op0=mybir.AluOpType.max)
                else:
                    for m in range(2):
                        nc.vector.tensor_scalar(
                            out=h1[:, ds(m * TOK, n)],
                            in0=p1[:, ds(m * TOK, n)], scalar1=0.0,
                            scalar2=None, op0=mybir.AluOpType.max)
                stage2.append((t, h1))

                if len(stage3) >= 2:
                    do_ep(*stage3.pop(0))

            while stage2:
                t2, h1 = stage2.pop(0)
                stage3.append((t2, do_l2(t2, h1)))
            while stage3:
                do_ep(*stage3.pop(0))

    nc.compile()
    return nc


def _get_compiled(n_full, rem):
    key = (n_full, rem)
    nc = _cache.get(key)
    if nc is None:
        nc = _build(n_full, rem)
        _cache[key] = nc
    return nc


def kernel(x, o, W1, b1, W2, b2, Wm, bm, Ws, bs):
    global LAST_RESULT
    from concourse import bass_utils
    import ml_dtypes

    x = np.asarray(x, dtype=np.float32)
    o_i = np.asarray(o).astype(np.int64)
    W1 = np.asarray(W1, dtype=np.float32)
    b1 = np.asarray(b1, dtype=np.float32)
    W2 = np.asarray(W2, dtype=np.float32)
    b2 = np.asarray(b2, dtype=np.float32)
    Wm = np.asarray(Wm, dtype=np.float32)
    bm = np.asarray(bm, dtype=np.float32)
    Ws = np.asarray(Ws, dtype=np.float32)
    bs = np.asarray(bs, dtype=np.float32)

    nb, d = x.shape
    counts = np.bincount(o_i, minlength=E)
    cmax = int(counts.max())
    n_full = max(1, cmax // TOK)
    rem = -(-max(0, cmax - n_full * TOK) // 16) * 16
    npad = n_full * TOK + rem
    order = np.argsort(o_i, kind="stable")
    idx_per_e = np.split(order, np.cumsum(counts)[:-1])
    sizes, offs = _sizes(n_full, rem)

    in_maps = []
    for e in range(E):
        idx = idx_per_e[e]
        xg = np.zeros((npad, DPAD), ml_dtypes.bfloat16)
        xg[:len(idx), :d] = x[idx].astype(ml_dtypes.bfloat16)
        xg[:, d] = 1.0            # ones-row: carries b1 through the matmul
        x_pack = np.concatenate(
            [xg[off:off + n].reshape(n, 3, 128).transpose(2, 1, 0).reshape(
                128, 3 * n) for off, n in zip(offs, sizes)], axis=1)
        x_pack = np.ascontiguousarray(x_pack)

        w1p = np.zeros((DPAD, H), np.float32)
        w1p[:d] = W1[e]
        w1p[d] = b1[e]
        w1_pack = np.ascontiguousarray(
            w1p.reshape(3, 128, H).transpose(1, 0, 2)).reshape(128, 3 * H)
        w2_pack = np.ascontiguousarray(
            W2[e].reshape(2, 128, H).transpose(1, 0, 2)).reshape(128, 2 * H)
        wc_full = np.zeros((H, 128), np.float32)
        wc_full[:, 0:A] = Wm[e]
        wc_full[:, 32:32 + A] = Ws[e]
        wc_pack = np.ascontiguousarray(
            wc_full.reshape(2, 128, 128).transpose(1, 0, 2)).reshape(128, 256)
        b2_pack = np.ascontiguousarray(b2[e].reshape(2, 128).T)
        bc_pack = np.zeros((128, 1), np.float32)
        bc_pack[32:32 + A, 0] = bs[e]
        rest_pack = np.concatenate(
            [w2_pack, wc_pack], axis=1).astype(ml_dtypes.bfloat16)
        bias_pack = np.concatenate([b2_pack, bc_pack], axis=1)

        in_maps.append({"x": x_pack, "w1": w1_pack.astype(ml_dtypes.bfloat16),
                        "rest": rest_pack, "bias": bias_pack})

    nc = _get_compiled(n_full, rem)

    kwargs = {}
    if TRACE:
        _install_axon_ntff_hook()
        bass_utils.upload_artifacts = lambda tmpdir: f"local:{tmpdir}"
        kwargs["trace"] = True
        if TRACE_CORES is not None:
            kwargs["trace_cores"] = TRACE_CORES
    res = None
    for attempt in range(3):
        try:
            res = bass_utils.run_bass_kernel_spmd(
                nc, in_maps, core_ids=list(range(E)), **kwargs)
            break
        except Exception:
            if attempt == 2:
                raise
            import time
            time.sleep(15)
    LAST_RESULT = res

    mean = np.empty((nb, A), np.float32)
    log_std = np.empty((nb, A), np.float32)
    for e in range(E):
        out = np.asarray(res.results[e]["out"], dtype=np.float32)  # [34, npad]
        ofull = out.T
        idx = idx_per_e[e]
        mean[idx] = ofull[:len(idx), :A] + bm[e]
        log_std[idx] = 3.5 * ofull[:len(idx), A:AOUT] - 1.5
    return mean, log_std
